# revision 62
# baseline (speedup 1.0000x reference)
"""Distributed Trainium2 kernel for a 16-head attention layer.

Problem: B=2, L=2048, HID=1024, H=16 (torch-Linear projections, masked softmax).
Sharding: 8 cores = batch (2) x query-chunk (4). The mask zeroes ~half the keys
uniformly across queries/heads (shape [B,1,1,L]), and exp(s-10000) underflows
to exactly 0 in f32, so masked keys are dropped host-side: keys are compacted
to NV valid keys padded to LT_K tiles of 128 (pad columns get a -10000 bias).

K/V projections are sharded across the 4-core batch group (each core projects
TL key-tiles) and exchanged with two AllGathers (K first so score matmuls can
start while V is still in flight). Per-core work: Q projection for its 512
queries, attention over all 16 heads x LT_K key tiles, output projection.

Attention layout (contraction dims on SBUF partitions throughout):
  scores are computed transposed (S.T[lk, lq]) so the pad-key bias is a
  per-partition bias fused into the ScalarE exp, and P.T feeds PV directly.
  PV packs the two heads of a pair into one PSUM bank via column-tiled
  matmuls (out partitions 0:64 / 64:128 run concurrently on the PE array).
  Softmax denominators come from ones-column stationary matmuls accumulated
  into PSUM partitions {0,32,64,96} (4 col-groups run concurrently), then one
  [4, 512] reciprocal + GpSimd partition broadcasts normalize each pair.
"""

import math
import sys
import types

import numpy as np
import ml_dtypes

# ---- problem constants (hardcoded; kernel.py must be self-contained) ----
B, L, HID, H = 2, 2048, 1024, 16
DH = HID // H          # 64
N_CORES = 8
GROUP = N_CORES // B   # 4 cores per batch group
LQ = (B * L) // N_CORES  # 512 queries per core
P = 128
KT = HID // P          # 8 contraction tiles
OT = HID // P          # 8 output tiles
NPAIR = H // 2         # 8 head pairs
SCALE = DH ** -0.5
BF16 = ml_dtypes.bfloat16

USE_AG = False         # in-group K/V AllGather: collective latency (~30us
                       # floor + slow transfer) exceeds the duplicated-
                       # projection compute it saves in this runtime


def _ensure_profile_hook():
    """Install the NTFF profiling hook trn_boot couldn't (antenv.axon_hooks
    is missing from the image); harmless if profiling is never requested."""
    if "antenv.axon_hooks" in sys.modules:
        return
    try:
        from trn_agent_boot.trn_boot import _ntff_profile_via_ctypes

        hook = _ntff_profile_via_ctypes("/opt/axon/libaxon_pjrt.so")
    except Exception:
        hook = None
    mod = types.ModuleType("antenv.axon_hooks")
    mod.get_axon_ntff_profile_hook = lambda: hook
    mod.set_axon_ntff_profile_hook = lambda h: None
    sys.modules["antenv.axon_hooks"] = mod


def build_bass(lt_k, tl, use_ag):
    """Build + compile the per-core Bass program (same graph on all 8 cores).

    lt_k: number of 128-key tiles attention iterates over (compacted+padded).
    tl:   K/V-projection key tiles computed locally per core (= lt_k if no AG).
    """
    import concourse.mybir as mybir
    import concourse.tile as tile
    from concourse import bacc

    f32 = mybir.dt.float32
    bf16 = mybir.dt.bfloat16
    ADD = mybir.AluOpType.add
    MULT = mybir.AluOpType.mult
    EXP = mybir.ActivationFunctionType.Exp

    LK = lt_k * P          # attention key width
    TLC = tl * P           # locally projected key width
    RG = [[0, 1, 2, 3], [4, 5, 6, 7]]

    nc = bacc.Bacc("TRN2", target_bir_lowering=False, debug=False, num_devices=N_CORES)

    qT = nc.declare_dram_parameter("qT", [HID, LQ], bf16, isOutput=False)
    kTl = nc.declare_dram_parameter("kTl", [HID, TLC], bf16, isOutput=False)
    vTl = nc.declare_dram_parameter("vTl", [HID, TLC], bf16, isOutput=False)
    WqT = nc.declare_dram_parameter("WqT", [HID, HID], bf16, isOutput=False)
    WkT = nc.declare_dram_parameter("WkT", [HID, HID], bf16, isOutput=False)
    WvT = nc.declare_dram_parameter("WvT", [HID, HID], bf16, isOutput=False)
    WoT = nc.declare_dram_parameter("WoT", [HID, HID], bf16, isOutput=False)
    bq = nc.declare_dram_parameter("bq", [P, OT], f32, isOutput=False)
    bk = nc.declare_dram_parameter("bk", [P, OT], f32, isOutput=False)
    bo = nc.declare_dram_parameter("bo", [P, OT], f32, isOutput=False)
    bv_row = nc.declare_dram_parameter("bv_row", [1, HID], bf16, isOutput=False)
    maskb = nc.declare_dram_parameter("maskb", [P, lt_k], f32, isOutput=False)
    out = nc.declare_dram_parameter("out", [HID, LQ], f32, isOutput=True)

    with tile.TileContext(nc) as tc:
        with (
            tc.tile_pool(name="consts", bufs=1) as consts,
            tc.tile_pool(name="khT", bufs=OT) as khT_p,
            tc.tile_pool(name="vhx", bufs=lt_k) as vhx_p,
            tc.tile_pool(name="qhT", bufs=OT) as qhT_p,
            tc.tile_pool(name="attnT", bufs=NPAIR) as attnT_p,
            tc.tile_pool(name="pt", bufs=12) as pt_p,
            tc.tile_pool(name="rec", bufs=1) as rec_p,
            tc.tile_pool(name="wq", bufs=KT) as wq_p,
            tc.tile_pool(name="qTin", bufs=KT) as qT_p,
            tc.tile_pool(name="wo", bufs=KT) as wo_p,
            tc.tile_pool(name="osb", bufs=2) as osb_p,
            tc.tile_pool(name="psum", bufs=1, space="PSUM") as psum,
            tc.tile_pool(name="dram", bufs=1, space="DRAM") as dram,
        ):
            # ---- constants (DMAs deferred until after the first K-proj
            # weight tiles so they don't delay the first matmul) ----
            ones_col = consts.tile([P, 1], bf16, tag="ones_col")
            nc.vector.memset(ones_col[:], 1.0)
            ones_row = consts.tile([1, P], bf16, tag="ones_row")
            nc.vector.memset(ones_row[:], 1.0)
            # ones on every partition, for the per-pair recip row broadcasts
            ones_rb = consts.tile([P, DH], bf16, tag="ones_rb")
            nc.vector.memset(ones_rb[:], 1.0)

            if use_ag:
                # one combined K+V bounce: K as [OT*P, TLC] rows, V appended as
                # [TLC, HID] viewed through a (r c) w -> r (c w) rearrange.
                assert HID % TLC == 0
                CVW = HID // TLC
                KROWS = OT * P + HID  # K rows + V bytes expressed at width TLC
                VR0 = (OT * P) // CVW
                agkv_in = dram.tile([KROWS, TLC], bf16, name="agkv_in")
                agkv_out = dram.tile([GROUP * KROWS, TLC], bf16, name="agkv_out")

            # ---- K projection (local tl tiles): khT_loc[ot] [128, TLC] ----
            khT = []
            with (
                tc.tile_pool(name="wk", bufs=KT) as wk_p,
                tc.tile_pool(name="kin", bufs=KT) as kin_p,
                tc.tile_pool(name="khl", bufs=OT) as khl_p,
            ):
                wk_sb = []
                kin_sb = []
                for i in range(KT):
                    w = wk_p.tile([P, HID], bf16, tag="wk")
                    nc.sync.dma_start(w[:], WkT[i * P : (i + 1) * P, :])
                    wk_sb.append(w)
                    x = kin_p.tile([P, TLC], bf16, tag="kin")
                    nc.sync.dma_start(x[:], kTl[i * P : (i + 1) * P, :])
                    kin_sb.append(x)
                    if i == 0:
                        # bk feeds the first K-proj bias — keep it right
                        # behind the first weight tile on the DMA queue
                        bk_sb = consts.tile([P, OT], f32, tag="bk")
                        nc.sync.dma_start(bk_sb[:], bk[:])
                maskb_sb = consts.tile([P, lt_k], f32)
                nc.sync.dma_start(maskb_sb[:], maskb[:])
                bq_sb = consts.tile([P, OT], f32, tag="bq")
                nc.sync.dma_start(bq_sb[:], bq[:])
                bo_sb = consts.tile([P, OT], f32, tag="bo")
                nc.sync.dma_start(bo_sb[:], bo[:])
                bv_sb = consts.tile([1, HID], bf16, tag="bvr")
                nc.sync.dma_start(bv_sb[:], bv_row[:])
                for ot in range(OT):
                    if use_ag:
                        t = khl_p.tile([P, TLC], bf16, tag="khl", name=f"khl{ot}")
                    else:
                        t = khT_p.tile([P, LK], bf16, tag="khT", name=f"khT{ot}")
                    for c0 in range(0, TLC, 512):
                        cw = min(512, TLC - c0)
                        ps = psum.tile([P, 1024], f32, tag="mm", bufs=2)
                        for i in range(KT):
                            nc.tensor.matmul(
                                ps[:, 0:cw],
                                wk_sb[i][:, ot * P : (ot + 1) * P],
                                kin_sb[i][:, c0 : c0 + cw],
                                start=(i == 0),
                                stop=(i == KT - 1),
                            )
                        nc.vector.tensor_scalar(
                            t[:, c0 : c0 + cw],
                            ps[:, 0:cw],
                            bk_sb[:, ot : ot + 1],
                            None,
                            op0=ADD,
                        )
                    if use_ag:
                        nc.sync.dma_start(agkv_in[ot * P : (ot + 1) * P, :], t[:])
                    else:
                        khT.append(t)
                # v-bias broadcast tile [128, HID] via rank-1 ones matmul
                # (emitted after K-proj so it fills the K->V transition and
                # never heads the PE queue waiting on the bv_row DMA)
                bvb_ps = psum.tile([P, 1024], f32, tag="mm", bufs=2)
                for h2 in range(2):
                    nc.tensor.matmul(
                        bvb_ps[:, h2 * 512 : (h2 + 1) * 512],
                        ones_row[:, :],
                        bv_sb[:, h2 * 512 : (h2 + 1) * 512],
                        start=True,
                        stop=True,
                    )
                bvb = consts.tile([P, HID], f32, tag="bvb")
                nc.vector.tensor_copy(bvb[:], bvb_ps[:])

            # ---- V projection (local tl tiles): vh [128 keys, 1024 feats] ----
            vhx = []
            with (
                tc.tile_pool(name="wv", bufs=KT) as wv_p,
                tc.tile_pool(name="vin", bufs=KT) as vin_p,
                tc.tile_pool(name="vhl", bufs=max(tl, 1)) as vhl_p,
            ):
                wv_sb = []
                vin_sb = []
                for i in range(KT):
                    w = wv_p.tile([P, HID], bf16, tag="wv")
                    nc.sync.dma_start(w[:], WvT[i * P : (i + 1) * P, :])
                    wv_sb.append(w)
                    x = vin_p.tile([P, TLC], bf16, tag="vin")
                    nc.sync.dma_start(x[:], vTl[i * P : (i + 1) * P, :])
                    vin_sb.append(x)
                for jl in range(tl):
                    ps = psum.tile([P, 1024], f32, tag="mm", bufs=2)
                    for half in range(2):
                        for i in range(KT):
                            nc.tensor.matmul(
                                ps[:, half * 512 : (half + 1) * 512],
                                vin_sb[i][:, jl * P : (jl + 1) * P],
                                wv_sb[i][:, half * 512 : (half + 1) * 512],
                                start=(i == 0),
                                stop=(i == KT - 1),
                            )
                    if use_ag:
                        t = vhl_p.tile([P, HID], bf16, tag="vhl", name=f"vhl{jl}")
                    else:
                        t = vhx_p.tile([P, HID], bf16, tag="vhx", name=f"vhx{jl}")
                    nc.vector.tensor_tensor(t[:], ps[:], bvb[:], op=ADD)
                    if use_ag:
                        vv = agkv_in[:].rearrange("(r c) w -> r (c w)", c=CVW)
                        nc.sync.dma_start(
                            vv[VR0 + jl * P : VR0 + (jl + 1) * P, :], t[:]
                        )
                    else:
                        vhx.append(t)

                if use_ag:
                    nc.gpsimd.collective_compute(
                        "AllGather",
                        mybir.AluOpType.bypass,
                        replica_groups=RG,
                        ins=[agkv_in.opt()],
                        outs=[agkv_out.opt()],
                    )

            # ---- Q projection: qhT[ot] [128, LQ] bf16. Only the first pair's
            # tiles are emitted up front; the rest interleave with attention
            # groups to fill the PE slack while ScalarE works through the exps.
            qhT = []
            wq_sb = []
            qT_sb = []
            for i in range(KT):
                w = wq_p.tile([P, HID], bf16, tag="wq")
                nc.sync.dma_start(w[:], WqT[i * P : (i + 1) * P, :])
                wq_sb.append(w)
                x = qT_p.tile([P, LQ], bf16, tag="qTin")
                nc.sync.dma_start(x[:], qT[i * P : (i + 1) * P, :])
                qT_sb.append(x)

            def emit_qproj(ot):
                ps = psum.tile([P, 1024], f32, tag="mm", bufs=2,
                               name=f"qps{ot}")
                for i in range(KT):
                    nc.tensor.matmul(
                        ps[:, 0:LQ],
                        wq_sb[i][:, ot * P : (ot + 1) * P],
                        qT_sb[i][:, :],
                        start=(i == 0),
                        stop=(i == KT - 1),
                    )
                t = qhT_p.tile([P, LQ], bf16, tag="qhT", name=f"qhT{ot}")
                nc.vector.tensor_scalar(
                    t[:], ps[:, 0:LQ], bq_sb[:, ot : ot + 1], None, op0=ADD
                )
                qhT.append(t)

            emit_qproj(0)
            emit_qproj(1)

            # ---- load gathered K/V from the AllGather outputs ----
            if use_ag:
                for ot in range(OT):
                    t = khT_p.tile([P, LK], bf16, tag="khT")
                    for q in range(GROUP):
                        c0 = q * TLC
                        if c0 >= LK:
                            break
                        cw = min(TLC, LK - c0)
                        r0 = q * KROWS + ot * P
                        nc.sync.dma_start(
                            t[:, c0 : c0 + cw], agkv_out[r0 : r0 + P, 0:cw]
                        )
                    khT.append(t)
                vvo = agkv_out[:].rearrange("(r c) w -> r (c w)", c=CVW)
                for j in range(lt_k):
                    q, jl = divmod(j, tl)
                    t = vhx_p.tile([P, HID], bf16, tag="vhx")
                    r0 = q * (KROWS // CVW) + VR0 + jl * P
                    nc.sync.dma_start(t[:], vvo[r0 : r0 + P, :])
                    vhx.append(t)

            # output-projection weights, loaded ahead so the tail can overlap
            wo_sb = []
            for i in range(KT):
                w = wo_p.tile([P, HID], bf16, tag="wo")
                nc.sync.dma_start(w[:], WoT[i * P : (i + 1) * P, :])
                wo_sb.append(w)
            o_ps = {}

            def emit_oproj_mms(ot, i_lo, i_hi):
                if ot not in o_ps:
                    o_ps[ot] = psum.tile([P, 1024], f32, tag="mm", bufs=2,
                                         name=f"ops{ot}")
                for i in range(i_lo, i_hi):
                    nc.tensor.matmul(
                        o_ps[ot][:, 0:LQ],
                        wo_sb[i][:, ot * P : (ot + 1) * P],
                        attnT[i][:, :],
                        start=(i == 0),
                        stop=(i == KT - 1),
                    )

            def emit_oproj_out(ot):
                o = osb_p.tile([P, LQ], f32, tag="osb", name=f"osb{ot}")
                nc.vector.tensor_scalar(
                    o[:], o_ps[ot][:, 0:LQ], bo_sb[:, ot : ot + 1], None, op0=ADD
                )
                nc.sync.dma_start(out[ot * P : (ot + 1) * P, :], o[:])

            # ---- attention: pairs in groups of 2, pipelined over j ----
            attnT = []
            pts = {}
            for g in range(NPAIR // 2):
                pairs = (2 * g, 2 * g + 1)
                pv = {}
                for hp in pairs:
                    pv[hp] = psum.tile([P, LQ], f32, tag="pv", bufs=2,
                                       name=f"pv{hp}")
                # 4 denominator rows accumulate into dnt col-groups {0,32,64,96}
                dnt = psum.tile([P, LQ], f32, tag="dn", bufs=1, name=f"dn{g}")
                # software-pipelined j-loop: QK+exp for step j are emitted with
                # PV+dn for step j-1, so the PE queue never waits on the exp
                # that produces the pt it is about to consume.
                def emit_qk_exp(hp, j):
                    sp = psum.tile([P, 1024], f32, tag="mm", bufs=2,
                                   name=f"sp{hp}_{j}")
                    nc.tensor.matmul(
                        sp[:, 0:512],
                        khT[hp][0:DH, j * P : (j + 1) * P],
                        qhT[hp][0:DH, :],
                        start=True,
                        stop=True,
                    )
                    nc.tensor.matmul(
                        sp[:, 512:1024],
                        khT[hp][DH:P, j * P : (j + 1) * P],
                        qhT[hp][DH:P, :],
                        start=True,
                        stop=True,
                    )
                    pt = pt_p.tile([P, 1024], bf16, tag="pt",
                                   name=f"pt{hp}_{j}")
                    nc.scalar.activation(
                        pt[:], sp[:], EXP,
                        bias=maskb_sb[:, j : j + 1], scale=SCALE,
                    )
                    pts[(hp, j)] = pt

                def emit_pv_dn(hp, j):
                    pt = pts.pop((hp, j))
                    # PV: two heads of the pair col-packed into one bank
                    nc.tensor.matmul(
                        pv[hp][0:DH, :],
                        vhx[j][:, (2 * hp) * DH : (2 * hp + 1) * DH],
                        pt[:, 0:512],
                        start=(j == 0),
                        stop=(j == lt_k - 1),
                    )
                    nc.tensor.matmul(
                        pv[hp][DH:P, :],
                        vhx[j][:, (2 * hp + 1) * DH : (2 * hp + 2) * DH],
                        pt[:, 512:1024],
                        start=(j == 0),
                        stop=(j == lt_k - 1),
                    )
                    # denominators: ones-column matmuls into PSUM rows
                    # {0,32,64,96} (distinct col-groups run concurrently)
                    ih = (hp - 2 * g) * 2
                    for half in range(2):
                        r = (ih + half) * 32
                        nc.tensor.matmul(
                            dnt[r : r + 1, :],
                            ones_col[:, :],
                            pt[:, half * 512 : (half + 1) * 512],
                            start=(j == 0),
                            stop=(j == lt_k - 1),
                            tile_position=(0, r),
                        )

                for j in range(lt_k):
                    for hp in pairs:
                        if (hp, j) not in pts:
                            emit_qk_exp(hp, j)
                    if j > 0:
                        for hp in pairs:
                            emit_pv_dn(hp, j - 1)
                # fill the wait on the final exps: emit the next pairs' Q
                # projection and pre-issue the next group's first QK+exp
                if g < NPAIR // 2 - 1:
                    emit_qproj(2 * g + 2)
                    emit_qproj(2 * g + 3)
                    for hp in (2 * g + 2, 2 * g + 3):
                        emit_qk_exp(hp, 0)
                for hp in pairs:
                    emit_pv_dn(hp, lt_k - 1)
                if g == NPAIR // 2 - 1:
                    # the last group's normalize chain is the tail: start the
                    # first two output tiles on the already-finished pairs
                    emit_oproj_mms(0, 0, 6)
                    emit_oproj_mms(1, 0, 6)
                # normalize: one wide reciprocal covers the group's denominator
                # rows {0,32,64} (+ row 0 of dnt2); junk rows in between are
                # never read (the broadcast matmuls contract only their row)
                dcp = rec_p.tile([P, LQ], f32, tag="dcp", bufs=2, name=f"dcp{g}")
                nc.vector.tensor_copy(dcp[0:97, :], dnt[0:97, :])
                rcf = rec_p.tile([P, LQ], f32, tag="rcf", bufs=2, name=f"rcf{g}")
                nc.vector.reciprocal_approx_fast(rcf[0:97, :], dcp[0:97, :])
                rcp = rec_p.tile([P, LQ], bf16, tag="rcp", bufs=2, name=f"rcp{g}")
                nc.vector.tensor_copy(rcp[0:97, :], rcf[0:97, :])
                for hp in pairs:
                    i0 = (hp - 2 * g) * 2
                    rbp = psum.tile([P, LQ], f32, tag="dn" if i0 == 0 else "dn2",
                                    bufs=1, name=f"rbp{hp}")
                    for half in range(2):
                        r = 32 * (i0 + half)
                        nc.tensor.matmul(
                            rbp[half * DH : (half + 1) * DH, :],
                            ones_rb[r : r + 1, :],
                            rcp[r : r + 1, :],
                            start=True,
                            stop=True,
                            tile_position=(r, half * DH),
                        )
                    rbs = rec_p.tile([P, LQ], f32, tag="rbs", bufs=2,
                                     name=f"rbs{hp}")
                    nc.vector.tensor_copy(rbs[:], rbp[:])
                    at = attnT_p.tile([P, LQ], bf16, tag="attnT", name=f"at{hp}")
                    nc.vector.tensor_tensor(at[:], pv[hp][:], rbs[:], op=MULT)
                    attnT.append(at)

            # ---- output projection: finish ot0/ot1, then the rest ----
            for ot in (0, 1):
                emit_oproj_mms(ot, 6, KT)
                emit_oproj_out(ot)
                del o_ps[ot]
            for ot in range(2, OT):
                emit_oproj_mms(ot, 0, KT)
                emit_oproj_out(ot)
                del o_ps[ot]

    nc.compile()
    return nc


def _plan(mask):
    """Compaction plan from the mask: valid key indices per batch + tiling."""
    mask = np.asarray(mask)
    idxs = [np.where(mask[b, 0, 0, :] != 0)[0] for b in range(B)]
    nv = max((len(ix) for ix in idxs), default=1)
    nv = max(nv, 1)
    lt_k = max(1, math.ceil(nv / P))
    tl = math.ceil(lt_k / GROUP) if USE_AG else lt_k
    return idxs, lt_k, tl


def make_in_maps(q, k, v, mask, Wq, bq, Wk, bk, Wv, bv, Wo, bo, idxs, lt_k, tl):
    """Shard + lay out the full inputs for the 8 cores (host-side numpy)."""
    q = np.asarray(q, np.float32)
    k = np.asarray(k, np.float32)
    v = np.asarray(v, np.float32)

    TLC = tl * P
    GW = (GROUP * TLC) if USE_AG else TLC  # compacted+padded key width

    def t_bf16(a):  # [R, C] -> contiguous [C, R] bf16
        return np.ascontiguousarray(np.asarray(a, np.float32).T).astype(BF16)

    WqT_h, WkT_h, WvT_h, WoT_h = (t_bf16(w) for w in (Wq, Wk, Wv, Wo))

    def b_tiles(b):  # [HID] -> [128, 8] f32 (per-o-tile partition vectors)
        return np.ascontiguousarray(np.asarray(b, np.float32).reshape(OT, P).T)

    bq_h, bk_h, bo_h = b_tiles(bq), b_tiles(bk), b_tiles(bo)
    bv_h = np.asarray(bv, np.float32)[None, :].astype(BF16)

    per_batch = {}
    for b in range(B):
        ix = idxs[b]
        nvb = len(ix)
        kc = np.zeros((GW, HID), np.float32)
        vc = np.zeros((GW, HID), np.float32)
        kc[:nvb] = k[b][ix]
        vc[:nvb] = v[b][ix]
        kcT = t_bf16(kc)  # [HID, GW]
        vcT = t_bf16(vc)
        mb = np.full(lt_k * P, -10000.0, np.float32)
        mb[:nvb] = 0.0
        maskb_h = np.ascontiguousarray(mb.reshape(lt_k, P).T)
        per_batch[b] = (kcT, vcT, maskb_h)

    in_maps = []
    for c in range(N_CORES):
        b, ch = divmod(c, GROUP)
        r0 = ch * LQ
        kcT, vcT, maskb_h = per_batch[b]
        if USE_AG:
            kTl_h = np.ascontiguousarray(kcT[:, ch * TLC : (ch + 1) * TLC])
            vTl_h = np.ascontiguousarray(vcT[:, ch * TLC : (ch + 1) * TLC])
        else:
            kTl_h, vTl_h = kcT, vcT
        in_maps.append(
            {
                "qT": t_bf16(q[b, r0 : r0 + LQ, :]),
                "kTl": kTl_h,
                "vTl": vTl_h,
                "WqT": WqT_h,
                "WkT": WkT_h,
                "WvT": WvT_h,
                "WoT": WoT_h,
                "bq": bq_h,
                "bk": bk_h,
                "bo": bo_h,
                "bv_row": bv_h,
                "maskb": maskb_h,
            }
        )
    return in_maps


def assemble_output(results):
    """Gather per-core out.T [HID, LQ] slices into the full [B, L, HID]."""
    full = np.empty((B, L, HID), np.float32)
    for c in range(N_CORES):
        b, ch = divmod(c, GROUP)
        r0 = ch * LQ
        full[b, r0 : r0 + LQ, :] = results[c]["out"].T
    return full


_NC_CACHE = {}


def _run(trace=False, **inputs):
    _ensure_profile_hook()
    from concourse.bass_utils import run_bass_kernel_spmd
    from concourse import bass_utils

    bass_utils.upload_artifacts = lambda tmpdir: tmpdir  # zero-egress container
    idxs, lt_k, tl = _plan(inputs["mask"])
    print(f"plan: nv={[len(ix) for ix in idxs]} lt_k={lt_k} tl={tl} use_ag={USE_AG}",
          flush=True)
    key = (lt_k, tl, USE_AG)
    if key not in _NC_CACHE:
        _NC_CACHE[key] = build_bass(lt_k, tl, USE_AG)
    in_maps = make_in_maps(
        **{k: v for k, v in inputs.items()}, idxs=idxs, lt_k=lt_k, tl=tl
    )
    res = run_bass_kernel_spmd(
        _NC_CACHE[key], in_maps, core_ids=list(range(N_CORES)), trace=trace
    )
    return assemble_output(res.results), res


def kernel(**inputs):
    out, _ = _run(trace=False, **inputs)
    return out
